# revision 4
# baseline (speedup 1.0000x reference)
"""Trainium2 Bass kernel for nn_CGCA_branch (gnn_message_passing).

Math: the reference applies 1x1 convs (C->CA, grouped CA->CA), global average
pool, fc1, adjacency-softmax matmul, relu, fc2, sigmoid.  Every op between x
and the relu is linear, and the global average pool commutes with the 1x1
convs, so the whole prefix collapses to

    f1[n, :] = Wcomb @ sum_s(x[n, :, s]),   Wcomb = fc1_w @ M2 @ (w1 / S)

with M2 the block-diagonal form of the grouped conv.  The kernel therefore
only needs a spatial-sum reduction of x (HBM-bound) plus tiny matmuls.

x is shipped to the device as float16: the spatial sums absorb the ~2^-11
per-element rounding (relative error ~1e-4 at the output, vs the 2e-2
tolerance), and halving the bytes halves the HBM stream time, which is the
whole kernel.

Layout: two consecutive channels per SBUF partition line, so each DMA
descriptor is a 12544-byte contiguous DRAM run - the size at which the 16
per-core DMA engines hit their ~26.7 GB/s-per-engine peak (6272-byte
descriptors measure ~25% slower).  Chunk = [128 partitions x (2ch x 3136)]
= 1.6 MB.

Reduction: chunks alternate between DVE and ACT so neither engine is the
bottleneck.  DVE uses the fused tensor_tensor_reduce (adds the two spatial
halves and reduces the result in one pass - half the cycles of a plain
reduce).  ACT uses activation-copy with the free accumulator.  Each engine's
chunks ride a dedicated DMA ring (SP HWDGE / Pool SWDGE) so one chain's
stalls can't head-of-line block the other; weights ride the third (ACT
HWDGE) ring, and all adjacency/softmax prep is emitted AFTER the stream
loop so no mid-stream engine ever waits on a tiny weight DMA queued behind
megabytes of x descriptors.

Sharding: pure data parallel - batch 64 split into 8 shards of 8 samples,
one per NeuronCore; weights replicated.
"""

import numpy as np

import concourse.bass as bass
import concourse.bacc as bacc
from concourse import mybir
from concourse.bass_utils import run_bass_kernel_spmd
from concourse.tile import TileContext
from contextlib import ExitStack

# ---- problem constants (hardcoded per harness contract) ----
N, C, H, W = 64, 512, 56, 56
S = H * W                      # 3136 spatial positions
J, CA, G = 17, 272, 16
NCORES = 8
NL = N // NCORES               # 8 samples per core
CT = C // 128                  # 4 channel chunks of 128 (last-sample layout)
CT2 = 2                        # 2 channel chunks of 256 (2ch-packed layout)
NEG = -9e15

_ADJ = np.array([
    [1,1,0,0,0,0,0,0,0,0,0,0,0,0,0,0,0],[1,1,1,0,0,0,0,0,0,0,0,0,0,0,0,0,0],
    [0,1,1,0,0,0,1,0,0,0,0,0,0,0,0,0,0],[0,0,0,1,1,0,1,0,0,0,0,0,0,0,0,0,0],
    [0,0,0,1,1,1,0,0,0,0,0,0,0,0,0,0,0],[0,0,0,0,1,1,0,0,0,0,0,0,0,0,0,0,0],
    [0,0,1,1,0,0,1,1,0,0,0,0,0,0,0,0,0],[0,0,0,0,0,0,1,1,1,0,0,0,0,0,0,0,0],
    [0,0,0,0,0,0,0,1,1,0,0,1,1,0,0,0,1],[0,0,0,0,0,0,0,0,0,1,0,0,0,0,0,0,1],
    [0,0,0,0,0,0,0,0,0,0,1,1,0,0,0,0,0],[0,0,0,0,0,0,0,0,0,0,1,1,1,0,0,0,0],
    [0,0,0,0,0,0,0,0,1,0,0,1,1,0,0,0,0],[0,0,0,0,0,0,0,0,1,0,0,0,0,1,1,0,0],
    [0,0,0,0,0,0,0,0,0,0,0,0,0,1,1,1,0],[0,0,0,0,0,0,0,0,0,0,0,0,0,0,1,1,0],
    [0,0,0,0,0,0,0,0,1,1,0,0,0,0,0,0,1]], dtype=np.int32)
NZ_IDX = np.flatnonzero(_ADJ)  # 49 entries

F32 = mybir.dt.float32
F16 = mybir.dt.float16
_NC_CACHE = {}


def _build_nc() -> bass.Bass:
    nc = bacc.Bacc(None, enable_partition_id=False)
    x_d = nc.declare_dram_parameter("x", [NL, C, S], F16, isOutput=False)
    # wct2[p, ct2, g, j] = Wcomb[j, ct2*256 + 2p + g]   (2ch-packed samples)
    wct2_d = nc.declare_dram_parameter("wct2", [128, CT2, 2, J], F32,
                                       isOutput=False)
    # wct1[p, ct, j] = Wcomb[j, ct*128 + p]             (last sample)
    wct1_d = nc.declare_dram_parameter("wct1", [128, CT, J], F32,
                                       isOutput=False)
    emat_d = nc.declare_dram_parameter("emat", [J, J], F32, isOutput=False)
    ematt_d = nc.declare_dram_parameter("ematt", [J, J], F32, isOutput=False)
    fc2t_d = nc.declare_dram_parameter("fc2t", [J, C], F32, isOutput=False)
    out_d = nc.declare_dram_parameter("out", [NL, C], F32, isOutput=True)

    with TileContext(nc) as tc, ExitStack() as ctx:
        xpool = ctx.enter_context(tc.tile_pool(name="xpool", bufs=8))
        singles = ctx.enter_context(tc.tile_pool(name="singles", bufs=1))
        smalls = ctx.enter_context(tc.tile_pool(name="smalls", bufs=3))
        resp = ctx.enter_context(tc.tile_pool(name="resp", bufs=1))
        psum = ctx.enter_context(tc.tile_pool(name="psum", bufs=2, space="PSUM"))

        # ---- replicated weights: ACT HWDGE ring, issued before any x DMA so
        # their descriptors sit at the front of the DMA-engine queues.
        wct2_sb = singles.tile([128, CT2, 2, J], F32)
        nc.scalar.dma_start(out=wct2_sb, in_=wct2_d[:, :, :, :])
        wct1_sb = singles.tile([128, CT, J], F32)
        nc.scalar.dma_start(out=wct1_sb, in_=wct1_d[:, :, :])
        fc2t_sb = singles.tile([J, C], F32)
        nc.scalar.dma_start(out=fc2t_sb, in_=fc2t_d[:, :])
        e_sb = singles.tile([J, J], F32)
        nc.scalar.dma_start(out=e_sb, in_=emat_d[:, :])
        et_sb = singles.tile([J, J], F32)
        nc.scalar.dma_start(out=et_sb, in_=ematt_d[:, :])

        # ---- stream x, spatial-sum per (sample, channel group) ----
        # xm2[p, ct2, g, n]: per-channel sums for the 2ch-packed samples 0-6;
        # xm1[p, ct]: sums for the last sample; stage: split-piece partials.
        xm2_sb = singles.tile([128, CT2, 2, NL - 1], F32)
        xm1_sb = singles.tile([128, CT], F32)
        stage = singles.tile([128, 12], F32)
        scratch = singles.tile([128, S], F16)           # dummy out for ACT accum
        tmp_sb = singles.tile([128, S // 2], F16)       # dummy out for DVE TTR
        f1_ps = psum.tile([J, NL], F32, tag="f1")       # f1 accumulator
        # c = ct2*256 + p*2 + q; (q s) is one contiguous 12544-byte DRAM run
        xv2 = x_d[:, :, :].rearrange("n (ct2 p q) s -> n p ct2 (q s)",
                                     ct2=CT2, p=128, q=2)
        xv1 = x_d[:, :, :].rearrange("n (ct p) s -> n p ct s", p=128)

        def dve_reduce(xt, w, dst):
            # fused (first-half + second-half) add + reduce: one DVE pass
            # over w/2 elements instead of w.
            h = w // 2
            nc.vector.scalar_tensor_tensor(
                out=tmp_sb[:, :h], in0=xt[:, 0:h], scalar=1.0, in1=xt[:, h:w],
                op0=mybir.AluOpType.mult, op1=mybir.AluOpType.add,
                accum_out=dst)

        def act_reduce(xt, w, dst):
            nc.scalar.activation(out=scratch[:, :w], in_=xt,
                                 func=mybir.ActivationFunctionType.Copy,
                                 accum_out=dst)

        # samples 0..6: 2ch-packed chunks, chunk0 -> DVE (SP ring),
        # chunk1 -> ACT (Pool SWDGE ring)
        f1_ops = {n: [] for n in range(NL)}
        for n in range(NL - 1):
            for ct2 in range(CT2):
                use_dve = ct2 == 0
                xt = xpool.tile([128, 2 * S], F16, tag="xt2")
                eng = nc.sync if use_dve else nc.gpsimd
                eng.dma_start(out=xt, in_=xv2[n, :, ct2, :])
                for g in range(2):
                    dst = xm2_sb[:, ct2, g, n:n + 1]
                    f1_ops[n].append((wct2_sb[:, ct2, g, :], dst))
                    if use_dve:
                        dve_reduce(xt[:, g * S:(g + 1) * S], S, dst)
                    else:
                        act_reduce(xt[:, g * S:(g + 1) * S], S, dst)
            for i, (lhsT, rhs) in enumerate(f1_ops[n]):
                nc.tensor.matmul(f1_ps[:, n:n + 1], lhsT=lhsT, rhs=rhs,
                                 start=(i == 0), stop=(i == 3))

        # last sample: 1ch chunks with a split tail so the final reduce after
        # the final DMA is short.  DVE takes ct1 + the 8-piece ct3 tail.
        n = NL - 1
        n_pieces = {2: 2, 3: 8}
        stage_col = 0
        for ct in range(CT):
            use_dve = ct % 2 == 1
            pieces = n_pieces.get(ct, 1)
            w = S // pieces
            for pi in range(pieces):
                xt = xpool.tile([128, w], F16, tag="xt1")
                eng = nc.sync if use_dve else nc.gpsimd
                eng.dma_start(out=xt, in_=xv1[n, :, ct, pi * w:(pi + 1) * w])
                if pieces == 1:
                    dst = xm1_sb[:, ct:ct + 1]
                else:
                    dst = stage[:, stage_col:stage_col + 1]
                    stage_col += 1
                f1_ops[n].append((wct1_sb[:, ct, :], dst))
                if use_dve:
                    dve_reduce(xt, w, dst)
                else:
                    act_reduce(xt, w, dst)
        for i, (lhsT, rhs) in enumerate(f1_ops[n]):
            nc.tensor.matmul(f1_ps[:, n:n + 1], lhsT=lhsT, rhs=rhs,
                             start=(i == 0), stop=(i == len(f1_ops[n]) - 1))

        # ---- adjacency softmax prep: emitted after the stream loop so the
        # in-order ACT/DVE queues never stall on it mid-stream.  adj[i,j] =
        # exp(E[i,j]) / rs[i]; exp(E^T) is the matmul lhsT, 1/rs folds in
        # after the matmul.
        a_sb = singles.tile([J, J], F32)
        nc.scalar.activation(out=a_sb, in_=e_sb,
                             func=mybir.ActivationFunctionType.Exp)
        at_sb = singles.tile([J, J], F32)
        nc.scalar.activation(out=at_sb, in_=et_sb,
                             func=mybir.ActivationFunctionType.Exp)
        rs_sb = singles.tile([J, 1], F32)
        nc.vector.reduce_sum(out=rs_sb, in_=a_sb, axis=mybir.AxisListType.X)
        rrs_sb = singles.tile([J, 1], F32)
        nc.vector.reciprocal(out=rrs_sb, in_=rs_sb)

        # ---- batched tail: gc = relu(adj @ f1); out = sigmoid(gc.T @ fc2t)
        f1_sb = smalls.tile([J, NL], F32, tag="f1s")
        nc.scalar.copy(out=f1_sb, in_=f1_ps)
        gc_ps = psum.tile([J, NL], F32, tag="gc")
        nc.tensor.matmul(gc_ps, lhsT=at_sb, rhs=f1_sb, start=True, stop=True)
        gc_sb = smalls.tile([J, NL], F32, tag="gcs")
        nc.vector.tensor_scalar(out=gc_sb, in0=gc_ps, scalar1=rrs_sb,
                                scalar2=0.0, op0=mybir.AluOpType.mult,
                                op1=mybir.AluOpType.max)
        res_sb = resp.tile([NL, C], F32, tag="res")
        half = C // 2
        for h in range(2):  # halves pipeline PE -> ACT -> DVE -> DMA
            o_ps = psum.tile([NL, half], F32, tag="o")
            nc.tensor.matmul(o_ps, lhsT=gc_sb,
                             rhs=fc2t_sb[:, h * half:(h + 1) * half],
                             start=True, stop=True)
            th_sb = smalls.tile([NL, half], F32, tag="th")
            nc.scalar.activation(out=th_sb, in_=o_ps,
                                 func=mybir.ActivationFunctionType.Tanh,
                                 scale=0.5)
            nc.vector.tensor_scalar(
                out=res_sb[:, h * half:(h + 1) * half], in0=th_sb,
                scalar1=0.5, scalar2=0.5, op0=mybir.AluOpType.mult,
                op1=mybir.AluOpType.add)
            nc.sync.dma_start(out=out_d[:, h * half:(h + 1) * half],
                              in_=res_sb[:, h * half:(h + 1) * half])

    return nc


def _get_nc() -> bass.Bass:
    if "nc" not in _NC_CACHE:
        nc = _build_nc()
        nc.finalize()
        _NC_CACHE["nc"] = nc
    return _NC_CACHE["nc"]


def _prep_inputs(x, e, w1, w2, fc1_w, fc2_w):
    """Host-side shard + weight fold (layout/precision prep only; the heavy
    math — reading and reducing all of x — happens on device)."""
    x = np.asarray(x, dtype=np.float32).reshape(N, C, S).astype(np.float16)

    # fold conv1 / grouped-conv2 / fc1 / (1/S mean) into one [J, C] matrix
    w1d = np.asarray(w1, dtype=np.float64)
    w2g = np.asarray(w2, dtype=np.float64).reshape(G, J, J)
    m2 = np.zeros((CA, CA), dtype=np.float64)
    for g in range(G):
        m2[g * J:(g + 1) * J, g * J:(g + 1) * J] = w2g[g]
    wcomb = np.asarray(fc1_w, np.float64) @ m2 @ (w1d / S)      # [J, C]
    # wct1[p, ct, j] = Wcomb[j, ct*128 + p]
    wct1 = np.ascontiguousarray(
        wcomb.T.reshape(CT, 128, J).transpose(1, 0, 2)).astype(np.float32)
    # wct2[p, ct2, g, j] = Wcomb[j, ct2*256 + 2p + g]
    wct2 = np.ascontiguousarray(
        wcomb.T.reshape(CT2, 128, 2, J).transpose(1, 0, 2, 3)
    ).astype(np.float32)

    emat = np.full((J * J,), NEG, dtype=np.float32)
    emat[NZ_IDX] = np.asarray(e, dtype=np.float32)[0]
    emat = emat.reshape(J, J)
    ematt = np.ascontiguousarray(emat.T)
    fc2t = np.ascontiguousarray(np.asarray(fc2_w, dtype=np.float32).T)

    in_maps = []
    for k in range(NCORES):
        in_maps.append({
            "x": np.ascontiguousarray(x[k * NL:(k + 1) * NL]),
            "wct2": wct2, "wct1": wct1, "emat": emat, "ematt": ematt,
            "fc2t": fc2t,
        })
    return in_maps


def _run(inputs: dict, trace: bool = False, trace_cores=None):
    in_maps = _prep_inputs(**inputs)
    nc = _get_nc()
    res = run_bass_kernel_spmd(nc, in_maps, list(range(NCORES)), trace=trace,
                               trace_cores=trace_cores)
    out = np.concatenate([res.results[k]["out"] for k in range(NCORES)], axis=0)
    return out.reshape(N, C, 1, 1).astype(np.float32), res


def kernel(**inputs) -> np.ndarray:
    out, _ = _run(inputs, trace=False)
    return out
